# revision 7
# baseline (speedup 1.0000x reference)
"""Series decomposition: depthwise moving-average (box filter, W=25, replicate
padding) + remainder, data-parallel over batch across 8 NeuronCores.

Per core: x shard [4, 512, 4096] viewed as [2048, 4096] rows. For each
[128, 4096] tile, build a replicate-padded tile XP[128, 13+L+12], then compute
the sliding-window sum with a single DVE scan using the recurrence

    s[i] = s[i-1] + xp[i+12] - xp[i-13]

(tensor_tensor_scan: state = (data0 + state) - data1), scale by the filter
weight (1/25) on the scalar engine, and subtract from x for the remainder.
This is O(1) work per element instead of O(W), so the kernel is DMA-bound.
"""

import numpy as np

import concourse.bacc as bacc
import concourse.bass as bass
import concourse.mybir as mybir
from concourse.bass_utils import run_bass_kernel_spmd
from concourse.tile import TileContext

B, C, L, W = 32, 512, 4096, 25
PAD = W // 2  # 12
NCORES = 8
ROWS = (B // NCORES) * C  # 2048 rows per core
P = 128
NTILES = ROWS // P  # 16
LPAD = PAD + 1  # 13 left-pad cols (extra col feeds the scan's subtract lag)
XCOLS = LPAD + L + PAD  # 4121

FP32 = mybir.dt.float32


def build_nc(scale: float, rows: int = ROWS, l: int = L) -> bass.Bass:
    ntiles = rows // P
    xcols = LPAD + l + PAD
    nc = bacc.Bacc(trn_type="TRN2")
    x = nc.dram_tensor("x", [rows, l], FP32, kind="ExternalInput")
    trend = nc.dram_tensor("trend", [rows, l], FP32, kind="ExternalOutput")
    remainder = nc.dram_tensor("remainder", [rows, l], FP32, kind="ExternalOutput")

    with TileContext(nc) as tc:
        with tc.tile_pool(name="pool", bufs=3) as pool:
            for i in range(ntiles):
                rsl = slice(i * P, (i + 1) * P)
                xp = pool.tile([P, xcols], FP32, tag="xp")
                nc.sync.dma_start(out=xp[:, LPAD : LPAD + l], in_=x[rsl, :])
                # replicate ('edge') padding on both sides
                nc.vector.tensor_copy(
                    out=xp[:, 0:LPAD],
                    in_=xp[:, LPAD : LPAD + 1].to_broadcast((P, LPAD)),
                )
                nc.vector.tensor_copy(
                    out=xp[:, LPAD + l : xcols],
                    in_=xp[:, LPAD + l - 1 : LPAD + l].to_broadcast((P, PAD)),
                )
                # window sum at i=-1 plus the lagged element the first scan
                # step subtracts: sum of xp cols [-13..11] = XP[:, 0:25]
                init = pool.tile([P, 1], FP32, tag="init")
                nc.vector.tensor_reduce(
                    out=init[:, 0:1],
                    in_=xp[:, 0:W],
                    axis=mybir.AxisListType.X,
                    op=mybir.AluOpType.add,
                )
                s = pool.tile([P, l], FP32, tag="s", bufs=2)
                nc.vector.tensor_tensor_scan(
                    out=s[:, :],
                    data0=xp[:, W:xcols],
                    data1=xp[:, 0:l],
                    initial=init[:, 0:1],
                    op0=mybir.AluOpType.add,
                    op1=mybir.AluOpType.subtract,
                )
                t = pool.tile([P, l], FP32, tag="t")
                nc.scalar.mul(t[:, :], s[:, :], scale)
                r = pool.tile([P, l], FP32, tag="r")
                nc.vector.tensor_sub(out=r[:, :], in0=xp[:, LPAD : LPAD + l], in1=t[:, :])
                nc.sync.dma_start(out=trend[rsl, :], in_=t[:, :])
                nc.sync.dma_start(out=remainder[rsl, :], in_=r[:, :])
    nc.finalize()
    return nc


def kernel(x, weight):
    x = np.ascontiguousarray(np.asarray(x), dtype=np.float32)
    # frozen depthwise moving-average kernel: every tap is 1/W
    scale = float(np.asarray(weight).reshape(-1)[0])
    nc = build_nc(scale)
    shards = x.reshape(NCORES, ROWS, L)
    in_maps = [{"x": shards[c]} for c in range(NCORES)]
    out = run_bass_kernel_spmd(nc, in_maps, core_ids=list(range(NCORES)))
    trend = np.concatenate(
        [out.results[c]["trend"][None] for c in range(NCORES)], axis=0
    ).reshape(B, C, L)
    remainder = np.concatenate(
        [out.results[c]["remainder"][None] for c in range(NCORES)], axis=0
    ).reshape(B, C, L)
    return trend, remainder


# revision 8
# speedup vs baseline: 6.2568x; 6.2568x over previous
"""Series decomposition: depthwise moving-average (box filter, W=25, replicate
padding) + remainder, data-parallel over batch across 8 NeuronCores.

Per core: x shard [4, 512, 4096] viewed as [2048, 4096] rows. For each
[128, 4096] tile, build a replicate-padded tile XP[128, 13+L+12], then compute
the sliding-window sum with a single DVE scan using the recurrence

    s[i] = s[i-1] + xp[i+12] - xp[i-13]

(tensor_tensor_scan: state = (data0 + state) - data1), scale by the filter
weight (1/25) on the scalar engine, and subtract from x for the remainder.
This is O(1) work per element instead of O(W), so the kernel is DMA-bound.
"""

import numpy as np

import concourse.bacc as bacc
import concourse.bass as bass
import concourse.mybir as mybir
from concourse.bass_utils import run_bass_kernel_spmd
from concourse.tile import TileContext

B, C, L, W = 32, 512, 4096, 25
PAD = W // 2  # 12
NCORES = 8
ROWS = (B // NCORES) * C  # 2048 rows per core
P = 128
NTILES = ROWS // P  # 16
LPAD = PAD + 1  # 13 left-pad cols (extra col feeds the scan's subtract lag)
XCOLS = LPAD + L + PAD  # 4121

FP32 = mybir.dt.float32


def build_nc(scale: float, rows: int = ROWS, l: int = L) -> bass.Bass:
    ntiles = rows // P
    xcols = LPAD + l + PAD
    nc = bacc.Bacc(trn_type="TRN2")
    x = nc.dram_tensor("x", [rows, l], FP32, kind="ExternalInput")
    trend = nc.dram_tensor("trend", [rows, l], FP32, kind="ExternalOutput")
    remainder = nc.dram_tensor("remainder", [rows, l], FP32, kind="ExternalOutput")

    with TileContext(nc) as tc:
        with tc.tile_pool(name="pool", bufs=3) as pool:
            for i in range(ntiles):
                rsl = slice(i * P, (i + 1) * P)
                xp = pool.tile([P, xcols], FP32, tag="xp")
                nc.sync.dma_start(out=xp[:, LPAD : LPAD + l], in_=x[rsl, :])
                # replicate ('edge') padding on both sides
                nc.vector.tensor_copy(
                    out=xp[:, 0:LPAD],
                    in_=xp[:, LPAD : LPAD + 1].to_broadcast((P, LPAD)),
                )
                nc.vector.tensor_copy(
                    out=xp[:, LPAD + l : xcols],
                    in_=xp[:, LPAD + l - 1 : LPAD + l].to_broadcast((P, PAD)),
                )
                # window sum at i=-1 plus the lagged element the first scan
                # step subtracts: sum of xp cols [-13..11] = XP[:, 0:25]
                init = pool.tile([P, 1], FP32, tag="init")
                nc.vector.tensor_reduce(
                    out=init[:, 0:1],
                    in_=xp[:, 0:W],
                    axis=mybir.AxisListType.X,
                    op=mybir.AluOpType.add,
                )
                s = pool.tile([P, l], FP32, tag="s", bufs=2)
                nc.vector.tensor_tensor_scan(
                    out=s[:, :],
                    data0=xp[:, W:xcols],
                    data1=xp[:, 0:l],
                    initial=init[:, 0:1],
                    op0=mybir.AluOpType.add,
                    op1=mybir.AluOpType.subtract,
                )
                t = pool.tile([P, l], FP32, tag="t")
                nc.scalar.mul(t[:, :], s[:, :], scale)
                r = pool.tile([P, l], FP32, tag="r")
                nc.vector.tensor_sub(out=r[:, :], in0=xp[:, LPAD : LPAD + l], in1=t[:, :])
                nc.sync.dma_start(out=trend[rsl, :], in_=t[:, :])
                nc.sync.dma_start(out=remainder[rsl, :], in_=r[:, :])
    nc.finalize()
    return nc


def _probe_devices():
    """Touch every NeuronCore with a trivial computation. After a previous
    client exits with in-flight bass executions, the first bass exec from a
    fresh client can fail with NRT_EXEC_UNIT_UNRECOVERABLE; a plain jax
    computation resets the state."""
    try:
        import jax
        import jax.numpy as jnp

        for d in jax.devices():
            y = jax.device_put(np.ones((4, 4), np.float32), d)
            jnp.sum(y).block_until_ready()
    except Exception:
        pass


def kernel(x, weight):
    x = np.ascontiguousarray(np.asarray(x), dtype=np.float32)
    # frozen depthwise moving-average kernel: every tap is 1/W
    scale = float(np.asarray(weight).reshape(-1)[0])
    nc = build_nc(scale)
    shards = x.reshape(NCORES, ROWS, L)
    in_maps = [{"x": shards[c]} for c in range(NCORES)]
    _probe_devices()
    try:
        out = run_bass_kernel_spmd(nc, in_maps, core_ids=list(range(NCORES)))
    except Exception:
        _probe_devices()
        out = run_bass_kernel_spmd(nc, in_maps, core_ids=list(range(NCORES)))
    trend = np.concatenate(
        [out.results[c]["trend"][None] for c in range(NCORES)], axis=0
    ).reshape(B, C, L)
    remainder = np.concatenate(
        [out.results[c]["remainder"][None] for c in range(NCORES)], axis=0
    ).reshape(B, C, L)
    return trend, remainder
